# revision 11
# baseline (speedup 1.0000x reference)
"""Trainium2 Bass kernel for heterogeneous GNN message passing (HGC).

Design (8 NeuronCores, SPMD single program):
  - Two 4-core groups: cores 0-3 handle relations (lsl on lrn, scs/sls on scn),
    cores 4-7 handle (cc/cac/csc on cpt). Each relation is dest-node-sharded
    across its group; a core's shard is 48 tiles x 128 dest slots = 6144 rows.
  - Projection (masked-mean MLP-table matmul) computed on device per dest slot.
  - 3 GCN rounds per relation: dma_gather of source rows (bf16) from a
    group-replicated h buffer, one-hot-with-attr selection matrices built by
    fused tensor_scalar(is_equal, mult), PSUM-accumulated scatter matmuls
    (agg_T = msgs^T @ S), then agg_T^T @ W_rel + relu.
  - Group AllGather (DRAM) of the 6144-row shard after projection and rounds 1,2.
  - Metapath softmax attention + final assembly on host (0.1% of FLOPs).
"""

import sys
sys.path.insert(0, "/opt/trn_rl_repo")

import os
import numpy as np
import ml_dtypes

import os
import concourse.bass as bass
import concourse.mybir as mybir
import concourse.tile as tile
from concourse import bacc
from concourse.bass_utils import run_bass_kernel_spmd
from concourse.masks import make_identity

BF16 = np.float16  # fp16: same PE/DVE speed as bf16, 8x finer mantissa
P = 128
D = 128
NL, NS, NC_ = 20000, 2000, 8000
CL = 2000                      # init matrix columns
CPAD = 2048                    # padded to 16 chunks of 128
TILES = 48                     # dest tiles per core
SHARD = TILES * P              # 6144
NCORES = 8
GSIZE = 4
HGLOB = GSIZE * SHARD          # 24576
SEG = 50                       # max chunks per gather segment
POOL_EVERY = int(os.environ.get('KPOOL', '8'))  # 0 = all one-hots on DVE

# tile layout within a core's 48 tiles
#   group A (cores 0-3): lsl tiles 0..39, scs 40..43, sls 44..47
#   group B (cores 4-7): cc 0..15, cac 16..31, csc 32..47
T_LRN, T_SCN, T_CPT = 40, 4, 16


def _f32(x):
    return np.ascontiguousarray(x, dtype=np.float32)


def _assign_space(deg, ncores, ntiles):
    """Assign nodes of a space to (core, tile, slot) bins, snake-balanced by
    degree. Returns (core, tile, slot) int arrays indexed by node id."""
    n = deg.shape[0]
    nbins = ncores * ntiles
    order = np.argsort(-deg, kind="stable")
    core = np.empty(n, np.int32)
    tl = np.empty(n, np.int32)
    slot = np.empty(n, np.int32)
    fill = np.zeros(nbins, np.int32)
    pos = np.arange(n)
    rnd = pos // nbins
    k = pos % nbins
    b = np.where(rnd % 2 == 0, k, nbins - 1 - k)
    core[order] = (b % ncores).astype(np.int32)
    tl[order] = (b // ncores).astype(np.int32)
    # slot index = how many nodes previously placed in same bin
    sl = np.zeros(n, np.int32)
    binid = b
    # within each bin, slots in placement (rnd) order; rnd is increasing with pos
    for bb in range(nbins):
        m = binid == bb
        sl[np.flatnonzero(m)] = np.arange(m.sum(), dtype=np.int32)
    slot[order] = sl
    assert sl.max(initial=0) < P
    return core, tl, slot


def _pack_idx(arr_i16):
    """[E] -> [128, E//16] int16 in the 16-partition wrap, replicated 8x."""
    e = arr_i16.shape[0]
    a = arr_i16.reshape(e // 16, 16).T          # [16, e//16]
    return np.ascontiguousarray(np.tile(a, (8, 1)))


def _pack_pc(arr):
    """[E] -> [128, E//128]; edge e at [e % 128, e // 128]."""
    e = arr.shape[0]
    return np.ascontiguousarray(arr.reshape(e // P, P).T)


def _build_table(W1, b1, W2, b2):
    idx = np.arange(CL, dtype=np.float32)[:, None]
    t = np.maximum(idx @ _f32(W1) + _f32(b1), 0.0) @ _f32(W2) + _f32(b2)  # [CL, D]
    tab = np.zeros((CPAD, D + 1), np.float32)
    tab[:CL, :D] = t
    tab[:CL, D] = 1.0
    return tab


_PROGRAM_CACHE = {}


def _build_program(c_segs):
    """c_segs: list over 48 tiles of list of segment chunk-counts (compile-time).
    Returns (nc, names dict)."""
    stage = os.environ.get("KSTAGE", "full")
    key = (stage,) + tuple(tuple(s) for s in c_segs)
    if key in _PROGRAM_CACHE:
        return _PROGRAM_CACHE[key]

    sum_c = sum(sum(s) for s in c_segs)
    nc = bacc.Bacc("TRN2", target_bir_lowering=False, num_devices=NCORES)
    f32, bf16, i16 = mybir.dt.float32, mybir.dt.float16, mybir.dt.int16

    init_rows = nc.dram_tensor("init_rows", [SHARD, CPAD], f32, kind="ExternalInput")
    tab0 = nc.dram_tensor("tab0", [P, 16 * (D + 1)], bf16, kind="ExternalInput")
    tab1 = nc.dram_tensor("tab1", [P, 16 * (D + 1)], bf16, kind="ExternalInput")
    wts = nc.dram_tensor("wts", [P, TILES * D], bf16, kind="ExternalInput")
    idx_all = nc.dram_tensor("idx_all", [P, 8 * sum_c], i16, kind="ExternalInput")
    col_all = nc.dram_tensor("col_all", [P, sum_c], f32, kind="ExternalInput")
    attr_all = nc.dram_tensor("attr_all", [P, sum_c], f32, kind="ExternalInput")
    iota_in = nc.dram_tensor("iota_in", [P, P], bf16, kind="ExternalInput")
    h3_out = nc.dram_tensor("h3_out", [SHARD, D], f32, kind="ExternalOutput")

    with tile.TileContext(nc) as tc:
        with (
            tc.tile_pool(name="const", bufs=1) as constp,
            tc.tile_pool(name="proj", bufs=3) as projp,
            tc.tile_pool(name="meta", bufs=4) as metap,
            tc.tile_pool(name="gath", bufs=3) as gathp,
            tc.tile_pool(name="sel", bufs=3) as selp,
            tc.tile_pool(name="out", bufs=3) as outp,
            tc.tile_pool(name="psum", bufs=2, space="PSUM") as psump,
            tc.tile_pool(name="psum2", bufs=2, space="PSUM") as psump2,
            tc.tile_pool(name="dram", bufs=1, space="DRAM") as dramp,
        ):
            ident = constp.tile([P, P], bf16)
            make_identity(nc, ident[:])
            iota_sb = constp.tile([P, P], bf16)
            nc.sync.dma_start(iota_sb[:], iota_in[:])
            tab0_sb = constp.tile([P, 16, D + 1], bf16)
            nc.sync.dma_start(tab0_sb[:], tab0[:].rearrange("p (k n) -> p k n", k=16))
            tab1_sb = constp.tile([P, 16, D + 1], bf16)
            nc.sync.dma_start(tab1_sb[:], tab1[:].rearrange("p (k n) -> p k n", k=16))
            w_sb = constp.tile([P, TILES, D], bf16)
            nc.sync.dma_start(w_sb[:], wts[:].rearrange("p (t n) -> p t n", t=TILES))

            h_shard = dramp.tile([SHARD, D], bf16)
            hg = [dramp.tile([HGLOB, D], bf16, name=f"hg{k}", tag=f"hg{k}")
                  for k in range(3)]

            # ---------------- projection ----------------
            for t in range(TILES):
                x_sb = projp.tile([P, CPAD], f32, tag="x")
                nc.sync.dma_start(x_sb[:], init_rows[t * P:(t + 1) * P, :])
                mask_sb = projp.tile([P, CPAD], bf16, tag="mask")
                nc.scalar.activation(mask_sb[:], x_sb[:],
                                     mybir.ActivationFunctionType.Sign)
                maskt_sb = projp.tile([P, 16, P], bf16, tag="maskt")
                for k in range(16):
                    tp = psump2.tile([P, P], bf16, tag="tp")
                    nc.tensor.transpose(tp[:], mask_sb[:, k * P:(k + 1) * P], ident[:])
                    nc.vector.tensor_copy(maskt_sb[:, k, :], tp[:])
                po = psump.tile([P, D + 1], f32, tag="po")
                tab_sb = tab0_sb if t < T_LRN else tab1_sb
                for k in range(16):
                    nc.tensor.matmul(po[:], maskt_sb[:, k, :], tab_sb[:, k, :],
                                     start=(k == 0), stop=(k == 15))
                mx = projp.tile([P, 1], f32, tag="mx")
                nc.vector.tensor_scalar_max(mx[:], po[:, D:D + 1], 1.0)
                rc = projp.tile([P, 1], f32, tag="rc")
                nc.vector.reciprocal(rc[:], mx[:])
                h0_sb = projp.tile([P, D], bf16, tag="h0")
                nc.vector.tensor_scalar_mul(h0_sb[:], po[:, :D], rc[:])
                nc.sync.dma_start(h_shard[t * P:(t + 1) * P, :].opt(), h0_sb[:])
                if stage == "proj":
                    h0f = projp.tile([P, D], f32, tag="h0f")
                    nc.vector.tensor_scalar_mul(h0f[:], po[:, :D], rc[:])
                    nc.sync.dma_start(h3_out[t * P:(t + 1) * P, :], h0f[:])

            if stage not in ("proj",):
                nc.gpsimd.collective_compute(
                    "AllGather", mybir.AluOpType.bypass,
                    replica_groups=[[0, 1, 2, 3], [4, 5, 6, 7]],
                    ins=[h_shard.opt()], outs=[hg[0].opt()],
                )

            if stage == "ag":
                nc.gpsimd.dma_start(h3_out[:, :].rearrange("(a b) d -> a b d", b=P), hg[0][:SHARD, :].rearrange("(a b) d -> a b d", b=P))

            # ---------------- GCN rounds ----------------
            n_iter = {"proj": 0, "ag": 0, "gcn1": 1}.get(stage, 3)
            for it in range(n_iter):
                src = hg[it]
                off = 0
                for t in range(TILES):
                    segs = c_segs[t]
                    ctot = sum(segs)
                    agg = psump.tile([P, D], f32, tag="agg")
                    ci = 0
                    for s_i, cs in enumerate(segs):
                        idx_sb = metap.tile([P, 8 * SEG], i16, tag="idx")
                        nc.sync.dma_start(idx_sb[:, :8 * cs],
                                          idx_all[:, 8 * off:8 * (off + cs)])
                        col_sb = metap.tile([P, SEG], f32, tag="col")
                        nc.sync.dma_start(col_sb[:, :cs], col_all[:, off:off + cs])
                        attr_sb = metap.tile([P, SEG], f32, tag="attr")
                        nc.sync.dma_start(attr_sb[:, :cs], attr_all[:, off:off + cs])
                        msgs = gathp.tile([P, SEG, D], bf16, tag="msgs")
                        nc.gpsimd.dma_gather(
                            msgs[:, :cs, :], src.opt(), idx_sb[:, :8 * cs],
                            cs * P, cs * P, D,
                            single_packet=False,
                        )
                        s_sb = selp.tile([P, SEG, P], bf16, tag="s")
                        for c in range(cs):
                            eng = nc.gpsimd if (POOL_EVERY > 0 and (c % POOL_EVERY) == (POOL_EVERY - 1)) \
                                else nc.vector
                            eng.tensor_scalar(
                                s_sb[:, c, :], iota_sb[:],
                                col_sb[:, c:c + 1], attr_sb[:, c:c + 1],
                                mybir.AluOpType.is_equal, mybir.AluOpType.mult,
                            )
                        for c in range(cs):
                            nc.tensor.matmul(agg[:], msgs[:, c, :], s_sb[:, c, :],
                                             start=(ci == 0), stop=(ci == ctot - 1))
                            ci += 1
                        off += cs
                    aggt_sb = outp.tile([P, D], bf16, tag="aggt")
                    nc.vector.tensor_copy(aggt_sb[:], agg[:])
                    hn = psump2.tile([P, D], f32, tag="hn")
                    nc.tensor.matmul(hn[:], aggt_sb[:], w_sb[:, t, :],
                                     start=True, stop=True)
                    if it < n_iter - 1:
                        hn_sb = outp.tile([P, D], bf16, tag="hnb")
                        nc.scalar.activation(hn_sb[:], hn[:],
                                             mybir.ActivationFunctionType.Relu)
                        nc.sync.dma_start(h_shard[t * P:(t + 1) * P, :].opt(), hn_sb[:])
                    else:
                        hn_sf = outp.tile([P, D], f32, tag="hnf")
                        nc.scalar.activation(hn_sf[:], hn[:],
                                             mybir.ActivationFunctionType.Relu)
                        nc.sync.dma_start(h3_out[t * P:(t + 1) * P, :], hn_sf[:])
                if it < n_iter - 1:
                    nc.gpsimd.collective_compute(
                        "AllGather", mybir.AluOpType.bypass,
                        replica_groups=[[0, 1, 2, 3], [4, 5, 6, 7]],
                        ins=[h_shard.opt()], outs=[hg[it + 1].opt()],
                    )

    nc.compile()
    _PROGRAM_CACHE[key] = nc
    return nc


def kernel(**inp):
    # ---------------- host preprocessing ----------------
    rel_edges = {k: (np.asarray(inp[f"p_{k}_edge_index"]),
                     np.asarray(inp[f"p_{k}_edge_attr"])) for k in
                 ["lsl", "scs", "sls", "cc", "cac", "csc"]}

    def deg(space_n, rels):
        d = np.zeros(space_n, np.int64)
        for r in rels:
            np.add.at(d, rel_edges[r][0][1], 1)
        return d

    lrn_c, lrn_t, lrn_s = _assign_space(deg(NL, ["lsl"]), GSIZE, T_LRN)
    scn_c, scn_t, scn_s = _assign_space(deg(NS, ["scs", "sls"]), GSIZE, T_SCN)
    cpt_c, cpt_t, cpt_s = _assign_space(deg(NC_, ["cc", "cac", "csc"]), GSIZE, T_CPT)

    # relation -> (space assign, tile offset within 48, group: 0=A cores0-3, 1=B)
    rel_cfg = {
        "lsl": ((lrn_c, lrn_t, lrn_s), 0, 0),
        "scs": ((scn_c, scn_t, scn_s), T_LRN, 0),
        "sls": ((scn_c, scn_t, scn_s), T_LRN + T_SCN, 0),
        "cc": ((cpt_c, cpt_t, cpt_s), 0, 1),
        "cac": ((cpt_c, cpt_t, cpt_s), T_CPT, 1),
        "csc": ((cpt_c, cpt_t, cpt_s), 2 * T_CPT, 1),
    }

    # per core, per global tile: list of (src_pos, col, attr) arrays
    tile_edges = [[None] * TILES for _ in range(NCORES)]
    for r, ((sc, st, ss), toff, grp) in rel_cfg.items():
        ei, ea = rel_edges[r]
        row, col = np.asarray(ei[0]).astype(np.int64), np.asarray(ei[1]).astype(np.int64)
        ea = np.asarray(ea, np.float32)
        src_pos = (sc[row].astype(np.int64) * SHARD
                   + (toff + st[row]).astype(np.int64) * P + ss[row])
        assert src_pos.max() < 32768
        oc = sc[col]
        dt_ = st[col]
        dsl = ss[col].astype(np.float32)
        for c in range(GSIZE):
            core = c + (0 if grp == 0 else GSIZE)
            m = oc == c
            tt, sp, cl, at = dt_[m], src_pos[m], dsl[m], ea[m]
            osort = np.argsort(tt, kind="stable")
            tt, sp, cl, at = tt[osort], sp[osort], cl[osort], at[osort]
            ntl = {"lsl": T_LRN, "scs": T_SCN, "sls": T_SCN,
                   "cc": T_CPT, "cac": T_CPT, "csc": T_CPT}[r]
            bnd = np.searchsorted(tt, np.arange(ntl + 1))
            for lt in range(ntl):
                a, b = bnd[lt], bnd[lt + 1]
                tile_edges[core][toff + lt] = (sp[a:b].astype(np.int16),
                                               cl[a:b], at[a:b])
    # group A cores have no entries for... every tile is covered by construction.

    # chunk counts per tile: max over cores, >=1, split into segments of <=SEG
    c_need = np.zeros((NCORES, TILES), np.int32)
    for core in range(NCORES):
        for t in range(TILES):
            e = tile_edges[core][t]
            n = 0 if e is None else len(e[0])
            c_need[core, t] = (n + P - 1) // P
    c_tile = np.maximum(c_need.max(axis=0), 1)
    c_segs = []
    for t in range(TILES):
        c = int(c_tile[t])
        segs = []
        while c > 0:
            s = min(c, SEG)
            segs.append(s)
            c -= s
        c_segs.append(segs)

    # pack per-core metadata
    sum_c = int(c_tile.sum())
    idx_np = np.zeros((NCORES, P, 8 * sum_c), np.int16)
    col_np = np.zeros((NCORES, P, sum_c), np.float32)
    attr_np = np.zeros((NCORES, P, sum_c), np.float32)
    for core in range(NCORES):
        off = 0
        for t in range(TILES):
            c = int(c_tile[t])
            ne = c * P
            e = tile_edges[core][t]
            sp = np.zeros(ne, np.int16)
            cl = np.zeros(ne, np.float32)
            at = np.zeros(ne, np.float32)
            if e is not None and len(e[0]):
                k = len(e[0])
                sp[:k], cl[:k], at[:k] = e[0], e[1], e[2]
            idx_np[core, :, 8 * off:8 * (off + c)] = _pack_idx(sp)
            col_np[core, :, off:off + c] = _pack_pc(cl)
            attr_np[core, :, off:off + c] = _pack_pc(at)
            off += c

    # init rows per core (slot-ordered), and tables/weights per group
    mats = {"lrn": _f32(inp["lrn_init"]), "scn": _f32(inp["scn_init"]),
            "cpt": _f32(inp["cpt_init"])}
    tabs = {n: _build_table(inp[f"proj_{n}_W1"], inp[f"proj_{n}_b1"],
                            inp[f"proj_{n}_W2"], inp[f"proj_{n}_b2"])
            for n in ["lrn", "scn", "cpt"]}

    def pack_tab(tabf):  # [CPAD, D+1] -> [128, 16*(D+1)] bf16
        return np.ascontiguousarray(
            tabf.reshape(16, P, D + 1).transpose(1, 0, 2).reshape(P, -1)
        ).astype(BF16)

    init_np = np.zeros((NCORES, SHARD, CPAD), np.float32)

    def fill_block(space, assign, toff, grp, nt):
        sc, st, ss = assign
        n = mats[space].shape[0]
        rows = (toff + st) * P + ss
        for c in range(GSIZE):
            core = c + (0 if grp == 0 else GSIZE)
            m = sc == c
            init_np[core, rows[m], :CL] = mats[space][np.flatnonzero(m)]

    fill_block("lrn", rel_cfg["lsl"][0], 0, 0, T_LRN)
    fill_block("scn", rel_cfg["scs"][0], T_LRN, 0, T_SCN)
    fill_block("scn", rel_cfg["sls"][0], T_LRN + T_SCN, 0, T_SCN)
    fill_block("cpt", rel_cfg["cc"][0], 0, 1, T_CPT)
    fill_block("cpt", rel_cfg["cac"][0], T_CPT, 1, T_CPT)
    fill_block("cpt", rel_cfg["csc"][0], 2 * T_CPT, 1, T_CPT)

    wmap_a = ["lsl"] * T_LRN + ["scs"] * T_SCN + ["sls"] * T_SCN
    wmap_b = ["cc"] * T_CPT + ["cac"] * T_CPT + ["csc"] * T_CPT

    def pack_w(wmap):
        w = np.zeros((P, TILES * D), np.float32)
        for t, r in enumerate(wmap):
            w[:, t * D:(t + 1) * D] = _f32(inp[f"W_{r}"])
        return w.astype(BF16)

    w_a, w_b = pack_w(wmap_a), pack_w(wmap_b)
    iota = np.ascontiguousarray(
        np.broadcast_to(np.arange(P, dtype=np.float32), (P, P))).astype(BF16)

    tab_lrn, tab_scn, tab_cpt = (pack_tab(tabs[n]) for n in ["lrn", "scn", "cpt"])
    in_maps = []
    for core in range(NCORES):
        grp = 0 if core < GSIZE else 1
        in_maps.append(dict(
            init_rows=init_np[core],
            tab0=tab_lrn if grp == 0 else tab_cpt,
            tab1=tab_scn if grp == 0 else tab_cpt,
            wts=w_a if grp == 0 else w_b,
            idx_all=idx_np[core], col_all=col_np[core], attr_all=attr_np[core],
            iota_in=iota,
        ))

    nc = _build_program(c_segs)
    res = run_bass_kernel_spmd(nc, in_maps, core_ids=list(range(NCORES)))
    if os.environ.get("KTIME", "0") == "1":
        import time as _time
        times = []
        for _ in range(3):
            t0 = _time.time()
            run_bass_kernel_spmd(nc, in_maps, core_ids=list(range(NCORES)))
            times.append(_time.time() - t0)
        print(f"warm exec wall times: {[f'{t:.3f}' for t in times]} s", flush=True)
        global LAST_WALL_NS
        LAST_WALL_NS = min(times) * 1e9
    global LAST_RES
    LAST_RES = res
    h3 = np.stack([res.results[c]["h3_out"] for c in range(NCORES)])  # [8,SHARD,D]

    # ---------------- host postprocessing ----------------
    def unshard(assign, toff, grp, n):
        sc, st, ss = assign
        rows = (toff + st) * P + ss
        out = np.empty((n, D), np.float32)
        cores = sc + (0 if grp == 0 else GSIZE)
        out[:] = h3[cores, rows]
        return out

    out_lsl = unshard(rel_cfg["lsl"][0], 0, 0, NL)
    out_scs = unshard(rel_cfg["scs"][0], T_LRN, 0, NS)
    out_sls = unshard(rel_cfg["sls"][0], T_LRN + T_SCN, 0, NS)
    out_cc = unshard(rel_cfg["cc"][0], 0, 1, NC_)
    out_cac = unshard(rel_cfg["cac"][0], T_CPT, 1, NC_)
    out_csc = unshard(rel_cfg["csc"][0], 2 * T_CPT, 1, NC_)
    global LAST_RAW
    LAST_RAW = dict(lsl=out_lsl, scs=out_scs, sls=out_sls,
                    cc=out_cc, cac=out_cac, csc=out_csc)

    def att(embs, w, b):
        e = np.stack(embs)                                  # [Pp, N, D]
        s = np.einsum("pnd,d->np", e, _f32(w)) + np.float32(b)
        s = s - s.max(axis=1, keepdims=True)
        a = np.exp(s)
        a /= a.sum(axis=1, keepdims=True)
        return np.einsum("np,pnd->nd", a, e).astype(np.float32)

    fin_scn = att([out_scs, out_sls], inp["att_scn_w"], inp["att_scn_b"])
    fin_cpt = att([out_cc, out_cac, out_csc], inp["att_cpt_w"], inp["att_cpt_b"])
    return out_lsl, fin_scn, fin_cpt


# revision 12
# speedup vs baseline: 1.1450x; 1.1450x over previous
"""Trainium2 Bass kernel for heterogeneous GNN message passing (HGC).

Design (8 NeuronCores, SPMD single program):
  - Two 4-core groups: cores 0-3 handle relations (lsl on lrn, scs/sls on scn),
    cores 4-7 handle (cc/cac/csc on cpt). Each relation is dest-node-sharded
    across its group; a core's shard is 48 tiles x 128 dest slots = 6144 rows.
  - Projection (masked-mean MLP-table matmul) computed on device per dest slot.
  - 3 GCN rounds per relation: dma_gather of source rows (bf16) from a
    group-replicated h buffer, one-hot-with-attr selection matrices built by
    fused tensor_scalar(is_equal, mult), PSUM-accumulated scatter matmuls
    (agg_T = msgs^T @ S), then agg_T^T @ W_rel + relu.
  - Group AllGather (DRAM) of the 6144-row shard after projection and rounds 1,2.
  - Metapath softmax attention + final assembly on host (0.1% of FLOPs).
"""

import sys
sys.path.insert(0, "/opt/trn_rl_repo")

import os
import numpy as np
import ml_dtypes

import os
import concourse.bass as bass
import concourse.mybir as mybir
import concourse.tile as tile
from concourse import bacc
from concourse.bass_utils import run_bass_kernel_spmd
from concourse.masks import make_identity

BF16 = np.float16  # fp16: same PE/DVE speed as bf16, 8x finer mantissa
P = 128
D = 128
NL, NS, NC_ = 20000, 2000, 8000
CL = 2000                      # init matrix columns
CPAD = 2048                    # padded to 16 chunks of 128
TILES = 48                     # dest tiles per core
SHARD = TILES * P              # 6144
NCORES = 8
GSIZE = 4
HGLOB = GSIZE * SHARD          # 24576
SEG = 50                       # max chunks per gather segment
POOL_EVERY = int(os.environ.get('KPOOL', '0'))  # 0 = all one-hots on DVE (gpsimd queue is kept free for gather DGE)

# tile layout within a core's 48 tiles
#   group A (cores 0-3): lsl tiles 0..39, scs 40..43, sls 44..47
#   group B (cores 4-7): cc 0..15, cac 16..31, csc 32..47
T_LRN, T_SCN, T_CPT = 40, 4, 16


def _f32(x):
    return np.ascontiguousarray(x, dtype=np.float32)


def _assign_space(deg, ncores, ntiles):
    """Assign nodes of a space to (core, tile, slot) bins, snake-balanced by
    degree. Returns (core, tile, slot) int arrays indexed by node id."""
    n = deg.shape[0]
    nbins = ncores * ntiles
    order = np.argsort(-deg, kind="stable")
    core = np.empty(n, np.int32)
    tl = np.empty(n, np.int32)
    slot = np.empty(n, np.int32)
    fill = np.zeros(nbins, np.int32)
    pos = np.arange(n)
    rnd = pos // nbins
    k = pos % nbins
    b = np.where(rnd % 2 == 0, k, nbins - 1 - k)
    core[order] = (b % ncores).astype(np.int32)
    tl[order] = (b // ncores).astype(np.int32)
    # slot index = how many nodes previously placed in same bin
    sl = np.zeros(n, np.int32)
    binid = b
    # within each bin, slots in placement (rnd) order; rnd is increasing with pos
    for bb in range(nbins):
        m = binid == bb
        sl[np.flatnonzero(m)] = np.arange(m.sum(), dtype=np.int32)
    slot[order] = sl
    assert sl.max(initial=0) < P
    return core, tl, slot


def _pack_idx(arr_i16):
    """[E] -> [128, E//16] int16 in the 16-partition wrap, replicated 8x."""
    e = arr_i16.shape[0]
    a = arr_i16.reshape(e // 16, 16).T          # [16, e//16]
    return np.ascontiguousarray(np.tile(a, (8, 1)))


def _pack_pc(arr):
    """[E] -> [128, E//128]; edge e at [e % 128, e // 128]."""
    e = arr.shape[0]
    return np.ascontiguousarray(arr.reshape(e // P, P).T)


def _build_table(W1, b1, W2, b2):
    idx = np.arange(CL, dtype=np.float32)[:, None]
    t = np.maximum(idx @ _f32(W1) + _f32(b1), 0.0) @ _f32(W2) + _f32(b2)  # [CL, D]
    tab = np.zeros((CPAD, D + 1), np.float32)
    tab[:CL, :D] = t
    tab[:CL, D] = 1.0
    return tab


_PROGRAM_CACHE = {}


def _build_program(c_segs):
    """c_segs: list over 48 tiles of list of segment chunk-counts (compile-time).
    Returns (nc, names dict)."""
    stage = os.environ.get("KSTAGE", "full")
    key = (stage,) + tuple(tuple(s) for s in c_segs)
    if key in _PROGRAM_CACHE:
        return _PROGRAM_CACHE[key]

    sum_c = sum(sum(s) for s in c_segs)
    nc = bacc.Bacc("TRN2", target_bir_lowering=False, num_devices=NCORES)
    f32, bf16, i16 = mybir.dt.float32, mybir.dt.float16, mybir.dt.int16

    init_rows = nc.dram_tensor("init_rows", [SHARD, CPAD], f32, kind="ExternalInput")
    tab0 = nc.dram_tensor("tab0", [P, 16 * (D + 1)], bf16, kind="ExternalInput")
    tab1 = nc.dram_tensor("tab1", [P, 16 * (D + 1)], bf16, kind="ExternalInput")
    wts = nc.dram_tensor("wts", [P, TILES * D], bf16, kind="ExternalInput")
    idx_all = nc.dram_tensor("idx_all", [P, 8 * sum_c], i16, kind="ExternalInput")
    col_all = nc.dram_tensor("col_all", [P, sum_c], f32, kind="ExternalInput")
    attr_all = nc.dram_tensor("attr_all", [P, sum_c], f32, kind="ExternalInput")
    iota_in = nc.dram_tensor("iota_in", [P, P], bf16, kind="ExternalInput")
    h3_out = nc.dram_tensor("h3_out", [SHARD, D], f32, kind="ExternalOutput")

    with tile.TileContext(nc) as tc:
        with (
            tc.tile_pool(name="const", bufs=1) as constp,
            tc.tile_pool(name="proj", bufs=3) as projp,
            tc.tile_pool(name="meta", bufs=4) as metap,
            tc.tile_pool(name="gath", bufs=3) as gathp,
            tc.tile_pool(name="sel", bufs=3) as selp,
            tc.tile_pool(name="out", bufs=3) as outp,
            tc.tile_pool(name="psum", bufs=2, space="PSUM") as psump,
            tc.tile_pool(name="psum2", bufs=2, space="PSUM") as psump2,
            tc.tile_pool(name="dram", bufs=1, space="DRAM") as dramp,
        ):
            ident = constp.tile([P, P], bf16)
            make_identity(nc, ident[:])
            iota_sb = constp.tile([P, P], bf16)
            nc.sync.dma_start(iota_sb[:], iota_in[:])
            tab0_sb = constp.tile([P, 16, D + 1], bf16)
            nc.sync.dma_start(tab0_sb[:], tab0[:].rearrange("p (k n) -> p k n", k=16))
            tab1_sb = constp.tile([P, 16, D + 1], bf16)
            nc.sync.dma_start(tab1_sb[:], tab1[:].rearrange("p (k n) -> p k n", k=16))
            w_sb = constp.tile([P, TILES, D], bf16)
            nc.sync.dma_start(w_sb[:], wts[:].rearrange("p (t n) -> p t n", t=TILES))

            h_shard = dramp.tile([SHARD, D], bf16)
            hg = [dramp.tile([HGLOB, D], bf16, name=f"hg{k}", tag=f"hg{k}")
                  for k in range(3)]

            # ---------------- projection ----------------
            for t in range(TILES):
                x_sb = projp.tile([P, CPAD], f32, tag="x")
                nc.sync.dma_start(x_sb[:], init_rows[t * P:(t + 1) * P, :])
                mask_sb = projp.tile([P, CPAD], bf16, tag="mask")
                nc.scalar.activation(mask_sb[:], x_sb[:],
                                     mybir.ActivationFunctionType.Sign)
                maskt_sb = projp.tile([P, 16, P], bf16, tag="maskt")
                for k in range(16):
                    tp = psump2.tile([P, P], bf16, tag="tp")
                    nc.tensor.transpose(tp[:], mask_sb[:, k * P:(k + 1) * P], ident[:])
                    nc.vector.tensor_copy(maskt_sb[:, k, :], tp[:])
                po = psump.tile([P, D + 1], f32, tag="po")
                tab_sb = tab0_sb if t < T_LRN else tab1_sb
                for k in range(16):
                    nc.tensor.matmul(po[:], maskt_sb[:, k, :], tab_sb[:, k, :],
                                     start=(k == 0), stop=(k == 15))
                mx = projp.tile([P, 1], f32, tag="mx")
                nc.vector.tensor_scalar_max(mx[:], po[:, D:D + 1], 1.0)
                rc = projp.tile([P, 1], f32, tag="rc")
                nc.vector.reciprocal(rc[:], mx[:])
                h0_sb = projp.tile([P, D], bf16, tag="h0")
                nc.vector.tensor_scalar_mul(h0_sb[:], po[:, :D], rc[:])
                nc.sync.dma_start(h_shard[t * P:(t + 1) * P, :].opt(), h0_sb[:])
                if stage == "proj":
                    h0f = projp.tile([P, D], f32, tag="h0f")
                    nc.vector.tensor_scalar_mul(h0f[:], po[:, :D], rc[:])
                    nc.sync.dma_start(h3_out[t * P:(t + 1) * P, :], h0f[:])

            if stage not in ("proj",):
                nc.gpsimd.collective_compute(
                    "AllGather", mybir.AluOpType.bypass,
                    replica_groups=[[0, 1, 2, 3], [4, 5, 6, 7]],
                    ins=[h_shard.opt()], outs=[hg[0].opt()],
                )

            if stage == "ag":
                nc.gpsimd.dma_start(h3_out[:, :].rearrange("(a b) d -> a b d", b=P), hg[0][:SHARD, :].rearrange("(a b) d -> a b d", b=P))

            # ---------------- GCN rounds ----------------
            n_iter = {"proj": 0, "ag": 0, "gcn1": 1}.get(stage, 3)
            for it in range(n_iter):
                src = hg[it]
                off = 0
                for t in range(TILES):
                    segs = c_segs[t]
                    ctot = sum(segs)
                    agg = psump.tile([P, D], f32, tag="agg")
                    ci = 0
                    for s_i, cs in enumerate(segs):
                        idx_sb = metap.tile([P, 8 * SEG], i16, tag="idx")
                        nc.sync.dma_start(idx_sb[:, :8 * cs],
                                          idx_all[:, 8 * off:8 * (off + cs)])
                        col_sb = metap.tile([P, SEG], f32, tag="col")
                        nc.sync.dma_start(col_sb[:, :cs], col_all[:, off:off + cs])
                        attr_sb = metap.tile([P, SEG], f32, tag="attr")
                        nc.sync.dma_start(attr_sb[:, :cs], attr_all[:, off:off + cs])
                        msgs = gathp.tile([P, SEG, D], bf16, tag="msgs")
                        nc.gpsimd.dma_gather(
                            msgs[:, :cs, :], src.opt(), idx_sb[:, :8 * cs],
                            cs * P, cs * P, D,
                            single_packet=False,
                        )
                        s_sb = selp.tile([P, SEG, P], bf16, tag="s")
                        for c in range(cs):
                            eng = nc.gpsimd if (POOL_EVERY > 0 and (c % POOL_EVERY) == (POOL_EVERY - 1)) \
                                else nc.vector
                            eng.tensor_scalar(
                                s_sb[:, c, :], iota_sb[:],
                                col_sb[:, c:c + 1], attr_sb[:, c:c + 1],
                                mybir.AluOpType.is_equal, mybir.AluOpType.mult,
                            )
                        for c in range(cs):
                            nc.tensor.matmul(agg[:], msgs[:, c, :], s_sb[:, c, :],
                                             start=(ci == 0), stop=(ci == ctot - 1))
                            ci += 1
                        off += cs
                    aggt_sb = outp.tile([P, D], bf16, tag="aggt")
                    nc.vector.tensor_copy(aggt_sb[:], agg[:])
                    hn = psump2.tile([P, D], f32, tag="hn")
                    nc.tensor.matmul(hn[:], aggt_sb[:], w_sb[:, t, :],
                                     start=True, stop=True)
                    if it < n_iter - 1:
                        hn_sb = outp.tile([P, D], bf16, tag="hnb")
                        nc.scalar.activation(hn_sb[:], hn[:],
                                             mybir.ActivationFunctionType.Relu)
                        nc.sync.dma_start(h_shard[t * P:(t + 1) * P, :].opt(), hn_sb[:])
                    else:
                        hn_sf = outp.tile([P, D], f32, tag="hnf")
                        nc.scalar.activation(hn_sf[:], hn[:],
                                             mybir.ActivationFunctionType.Relu)
                        nc.sync.dma_start(h3_out[t * P:(t + 1) * P, :], hn_sf[:])
                if it < n_iter - 1:
                    nc.gpsimd.collective_compute(
                        "AllGather", mybir.AluOpType.bypass,
                        replica_groups=[[0, 1, 2, 3], [4, 5, 6, 7]],
                        ins=[h_shard.opt()], outs=[hg[it + 1].opt()],
                    )

    nc.compile()
    _PROGRAM_CACHE[key] = nc
    return nc


def kernel(**inp):
    # ---------------- host preprocessing ----------------
    rel_edges = {k: (np.asarray(inp[f"p_{k}_edge_index"]),
                     np.asarray(inp[f"p_{k}_edge_attr"])) for k in
                 ["lsl", "scs", "sls", "cc", "cac", "csc"]}

    def deg(space_n, rels):
        d = np.zeros(space_n, np.int64)
        for r in rels:
            np.add.at(d, rel_edges[r][0][1], 1)
        return d

    lrn_c, lrn_t, lrn_s = _assign_space(deg(NL, ["lsl"]), GSIZE, T_LRN)
    scn_c, scn_t, scn_s = _assign_space(deg(NS, ["scs", "sls"]), GSIZE, T_SCN)
    cpt_c, cpt_t, cpt_s = _assign_space(deg(NC_, ["cc", "cac", "csc"]), GSIZE, T_CPT)

    # relation -> (space assign, tile offset within 48, group: 0=A cores0-3, 1=B)
    rel_cfg = {
        "lsl": ((lrn_c, lrn_t, lrn_s), 0, 0),
        "scs": ((scn_c, scn_t, scn_s), T_LRN, 0),
        "sls": ((scn_c, scn_t, scn_s), T_LRN + T_SCN, 0),
        "cc": ((cpt_c, cpt_t, cpt_s), 0, 1),
        "cac": ((cpt_c, cpt_t, cpt_s), T_CPT, 1),
        "csc": ((cpt_c, cpt_t, cpt_s), 2 * T_CPT, 1),
    }

    # per core, per global tile: list of (src_pos, col, attr) arrays
    tile_edges = [[None] * TILES for _ in range(NCORES)]
    for r, ((sc, st, ss), toff, grp) in rel_cfg.items():
        ei, ea = rel_edges[r]
        row, col = np.asarray(ei[0]).astype(np.int64), np.asarray(ei[1]).astype(np.int64)
        ea = np.asarray(ea, np.float32)
        src_pos = (sc[row].astype(np.int64) * SHARD
                   + (toff + st[row]).astype(np.int64) * P + ss[row])
        assert src_pos.max() < 32768
        oc = sc[col]
        dt_ = st[col]
        dsl = ss[col].astype(np.float32)
        for c in range(GSIZE):
            core = c + (0 if grp == 0 else GSIZE)
            m = oc == c
            tt, sp, cl, at = dt_[m], src_pos[m], dsl[m], ea[m]
            osort = np.argsort(tt, kind="stable")
            tt, sp, cl, at = tt[osort], sp[osort], cl[osort], at[osort]
            ntl = {"lsl": T_LRN, "scs": T_SCN, "sls": T_SCN,
                   "cc": T_CPT, "cac": T_CPT, "csc": T_CPT}[r]
            bnd = np.searchsorted(tt, np.arange(ntl + 1))
            for lt in range(ntl):
                a, b = bnd[lt], bnd[lt + 1]
                tile_edges[core][toff + lt] = (sp[a:b].astype(np.int16),
                                               cl[a:b], at[a:b])
    # group A cores have no entries for... every tile is covered by construction.

    # chunk counts per tile: max over cores, >=1, split into segments of <=SEG
    c_need = np.zeros((NCORES, TILES), np.int32)
    for core in range(NCORES):
        for t in range(TILES):
            e = tile_edges[core][t]
            n = 0 if e is None else len(e[0])
            c_need[core, t] = (n + P - 1) // P
    c_tile = np.maximum(c_need.max(axis=0), 1)
    c_segs = []
    for t in range(TILES):
        c = int(c_tile[t])
        segs = []
        while c > 0:
            s = min(c, SEG)
            segs.append(s)
            c -= s
        c_segs.append(segs)

    # pack per-core metadata
    sum_c = int(c_tile.sum())
    idx_np = np.zeros((NCORES, P, 8 * sum_c), np.int16)
    col_np = np.zeros((NCORES, P, sum_c), np.float32)
    attr_np = np.zeros((NCORES, P, sum_c), np.float32)
    for core in range(NCORES):
        off = 0
        for t in range(TILES):
            c = int(c_tile[t])
            ne = c * P
            e = tile_edges[core][t]
            sp = np.zeros(ne, np.int16)
            cl = np.zeros(ne, np.float32)
            at = np.zeros(ne, np.float32)
            if e is not None and len(e[0]):
                k = len(e[0])
                sp[:k], cl[:k], at[:k] = e[0], e[1], e[2]
            idx_np[core, :, 8 * off:8 * (off + c)] = _pack_idx(sp)
            col_np[core, :, off:off + c] = _pack_pc(cl)
            attr_np[core, :, off:off + c] = _pack_pc(at)
            off += c

    # init rows per core (slot-ordered), and tables/weights per group
    mats = {"lrn": _f32(inp["lrn_init"]), "scn": _f32(inp["scn_init"]),
            "cpt": _f32(inp["cpt_init"])}
    tabs = {n: _build_table(inp[f"proj_{n}_W1"], inp[f"proj_{n}_b1"],
                            inp[f"proj_{n}_W2"], inp[f"proj_{n}_b2"])
            for n in ["lrn", "scn", "cpt"]}

    def pack_tab(tabf):  # [CPAD, D+1] -> [128, 16*(D+1)] bf16
        return np.ascontiguousarray(
            tabf.reshape(16, P, D + 1).transpose(1, 0, 2).reshape(P, -1)
        ).astype(BF16)

    init_np = np.zeros((NCORES, SHARD, CPAD), np.float32)

    def fill_block(space, assign, toff, grp, nt):
        sc, st, ss = assign
        n = mats[space].shape[0]
        rows = (toff + st) * P + ss
        for c in range(GSIZE):
            core = c + (0 if grp == 0 else GSIZE)
            m = sc == c
            init_np[core, rows[m], :CL] = mats[space][np.flatnonzero(m)]

    fill_block("lrn", rel_cfg["lsl"][0], 0, 0, T_LRN)
    fill_block("scn", rel_cfg["scs"][0], T_LRN, 0, T_SCN)
    fill_block("scn", rel_cfg["sls"][0], T_LRN + T_SCN, 0, T_SCN)
    fill_block("cpt", rel_cfg["cc"][0], 0, 1, T_CPT)
    fill_block("cpt", rel_cfg["cac"][0], T_CPT, 1, T_CPT)
    fill_block("cpt", rel_cfg["csc"][0], 2 * T_CPT, 1, T_CPT)

    wmap_a = ["lsl"] * T_LRN + ["scs"] * T_SCN + ["sls"] * T_SCN
    wmap_b = ["cc"] * T_CPT + ["cac"] * T_CPT + ["csc"] * T_CPT

    def pack_w(wmap):
        w = np.zeros((P, TILES * D), np.float32)
        for t, r in enumerate(wmap):
            w[:, t * D:(t + 1) * D] = _f32(inp[f"W_{r}"])
        return w.astype(BF16)

    w_a, w_b = pack_w(wmap_a), pack_w(wmap_b)
    iota = np.ascontiguousarray(
        np.broadcast_to(np.arange(P, dtype=np.float32), (P, P))).astype(BF16)

    tab_lrn, tab_scn, tab_cpt = (pack_tab(tabs[n]) for n in ["lrn", "scn", "cpt"])
    in_maps = []
    for core in range(NCORES):
        grp = 0 if core < GSIZE else 1
        in_maps.append(dict(
            init_rows=init_np[core],
            tab0=tab_lrn if grp == 0 else tab_cpt,
            tab1=tab_scn if grp == 0 else tab_cpt,
            wts=w_a if grp == 0 else w_b,
            idx_all=idx_np[core], col_all=col_np[core], attr_all=attr_np[core],
            iota_in=iota,
        ))

    nc = _build_program(c_segs)
    res = run_bass_kernel_spmd(nc, in_maps, core_ids=list(range(NCORES)))
    if os.environ.get("KTIME", "0") == "1":
        import time as _time
        times = []
        for _ in range(3):
            t0 = _time.time()
            run_bass_kernel_spmd(nc, in_maps, core_ids=list(range(NCORES)))
            times.append(_time.time() - t0)
        print(f"warm exec wall times: {[f'{t:.3f}' for t in times]} s", flush=True)
        global LAST_WALL_NS
        LAST_WALL_NS = min(times) * 1e9
    global LAST_RES
    LAST_RES = res
    h3 = np.stack([res.results[c]["h3_out"] for c in range(NCORES)])  # [8,SHARD,D]

    # ---------------- host postprocessing ----------------
    def unshard(assign, toff, grp, n):
        sc, st, ss = assign
        rows = (toff + st) * P + ss
        out = np.empty((n, D), np.float32)
        cores = sc + (0 if grp == 0 else GSIZE)
        out[:] = h3[cores, rows]
        return out

    out_lsl = unshard(rel_cfg["lsl"][0], 0, 0, NL)
    out_scs = unshard(rel_cfg["scs"][0], T_LRN, 0, NS)
    out_sls = unshard(rel_cfg["sls"][0], T_LRN + T_SCN, 0, NS)
    out_cc = unshard(rel_cfg["cc"][0], 0, 1, NC_)
    out_cac = unshard(rel_cfg["cac"][0], T_CPT, 1, NC_)
    out_csc = unshard(rel_cfg["csc"][0], 2 * T_CPT, 1, NC_)
    global LAST_RAW
    LAST_RAW = dict(lsl=out_lsl, scs=out_scs, sls=out_sls,
                    cc=out_cc, cac=out_cac, csc=out_csc)

    def att(embs, w, b):
        e = np.stack(embs)                                  # [Pp, N, D]
        s = np.einsum("pnd,d->np", e, _f32(w)) + np.float32(b)
        s = s - s.max(axis=1, keepdims=True)
        a = np.exp(s)
        a /= a.sum(axis=1, keepdims=True)
        return np.einsum("np,pnd->nd", a, e).astype(np.float32)

    fin_scn = att([out_scs, out_sls], inp["att_scn_w"], inp["att_scn_b"])
    fin_cpt = att([out_cc, out_cac, out_csc], inp["att_cpt_w"], inp["att_cpt_b"])
    return out_lsl, fin_scn, fin_cpt
